# revision 9
# baseline (speedup 1.0000x reference)
"""Trainium2 Bass kernel for nn_Denoiser (dense MLP 2->16->16x5->2, N=4194304).

Strategy (pure data parallel over 8 NeuronCores):
  - Shard the batch over 8 cores (524288 points each); weights replicated.
  - On each core, stack 8 batch groups along SBUF partitions and use
    block-diagonal weights so each fp32 matmul column carries 8 points
    (the 128x128 PE array runs 8 independent 16-wide MLPs at once).
  - Activations are [128, 1024] fp32 tiles (features x batch-columns);
    each layer is 2 matmuls (PSUM bank = 512 fp32 cols); the 6 inner
    ReLUs are fused into the PSUM->SBUF evacuation, split between
    VectorE (tensor_scalar_max) and ScalarE (activation Relu).
  - The final 16->2 layer accumulates 4 super-chunks into one packed
    [128, 1024] PSUM tile (weights block-shifted by 32 partitions per
    super-chunk) so a single PSUM->SBUF copy serves 4 super-chunks.
  - The host pre-permutes x into the exact per-partition layout
    ([16, S*C], partition = 2*group+feature) so every device DMA is a
    contiguous 2D slice; the packed output [128, (S/4)*C] is decoded on
    the host the same way.
"""

import numpy as np

N = 4194304
N_CORES = 8
N_SHARD = N // N_CORES  # 524288
G = 8          # batch groups stacked along partitions
C = 1024       # batch columns per super-chunk tile (2 PSUM banks)
S = N_SHARD // (G * C)  # 64 super-chunks per core
PACK = 4       # super-chunks packed per output evacuation
N_NODE = 16
N_INT = 5

_CACHE = {}

# Set by test harnesses: TRACE=True captures an NTFF profile; the
# BassKernelResults of the last run lands in LAST_RESULT.
TRACE = False
LAST_RESULT = None


def _build_bass():
    from contextlib import ExitStack

    import concourse.mybir as mybir
    import concourse.tile as tile
    from concourse import bacc

    f32 = mybir.dt.float32
    nc = bacc.Bacc("TRN2", target_bir_lowering=False, num_devices=N_CORES)

    # xd[2g+f, s*C + c] = x[s*G*C + g*C + c, f]   (host pre-permuted)
    xd = nc.dram_tensor("xd", [16, S * C], f32, kind="ExternalInput")
    w0 = nc.dram_tensor("w0", [16, 128], f32, kind="ExternalInput")
    wm = nc.dram_tensor("wm", [N_INT, 128, 128], f32, kind="ExternalInput")
    w6 = nc.dram_tensor("w6", [PACK, 128, 128], f32, kind="ExternalInput")
    # yd[32*(s%4)+2g+f, (s//4)*C + c] = y[s*G*C + g*C + c, f]
    yd = nc.dram_tensor("yd", [128, (S // PACK) * C], f32, kind="ExternalOutput")

    with tile.TileContext(nc) as tc, ExitStack() as ctx:
        wpool = ctx.enter_context(tc.tile_pool(name="weights", bufs=1))
        xpool = ctx.enter_context(tc.tile_pool(name="x", bufs=3))
        hpool = ctx.enter_context(tc.tile_pool(name="h", bufs=3))
        opool = ctx.enter_context(tc.tile_pool(name="o", bufs=2))
        pspool = ctx.enter_context(tc.tile_pool(name="ps", bufs=3, space="PSUM"))
        pkpool = ctx.enter_context(tc.tile_pool(name="pk", bufs=1, space="PSUM"))

        w0_t = wpool.tile([16, 128], f32, tag="w0")
        nc.sync.dma_start(out=w0_t, in_=w0[:, :])
        wm_t = []
        for l in range(N_INT):
            t = wpool.tile([128, 128], f32, tag=f"wm{l}")
            nc.sync.dma_start(out=t, in_=wm[l, :, :])
            wm_t.append(t)
        w6_t = []
        for j in range(PACK):
            t = wpool.tile([128, 128], f32, tag=f"w6{j}")
            nc.sync.dma_start(out=t, in_=w6[j, :, :])
            w6_t.append(t)

        pk_t = None
        for s in range(S):
            j = s % PACK
            x_t = xpool.tile([16, C], f32, tag="x")
            nc.sync.dma_start(out=x_t, in_=xd[:, s * C : (s + 1) * C])
            h = x_t
            for l in range(6):
                lhsT = w0_t if l == 0 else wm_t[l - 1]
                ps_t = pspool.tile([128, C], f32, tag="ps")
                for q in range(C // 512):
                    nc.tensor.matmul(
                        ps_t[:, 512 * q : 512 * (q + 1)],
                        lhsT,
                        h[:, 512 * q : 512 * (q + 1)],
                        start=True,
                        stop=True,
                    )
                h_new = hpool.tile([128, C], f32, tag="h")
                # ~44/56 DVE/ACT split balances the two evacuation engines.
                if (s + l) % 9 < 4:
                    nc.vector.tensor_scalar_max(h_new, ps_t, 0.0)
                else:
                    nc.scalar.activation(
                        h_new, ps_t, mybir.ActivationFunctionType.Relu
                    )
                h = h_new
            if j == 0:
                pk_t = pkpool.tile([128, C], f32, tag="pk")
            for q in range(C // 512):
                nc.tensor.matmul(
                    pk_t[:, 512 * q : 512 * (q + 1)],
                    w6_t[j],
                    h[:, 512 * q : 512 * (q + 1)],
                    start=(j == 0),
                    stop=(j == PACK - 1),
                    skip_group_check=True,
                )
            if j == PACK - 1:
                sp = s // PACK
                o_t = opool.tile([128, C], f32, tag="o")
                nc.any.tensor_copy(o_t, pk_t)
                nc.sync.dma_start(out=yd[:, sp * C : (sp + 1) * C], in_=o_t)
    nc.compile()
    return nc


def _prep_weights(w_in, w_mid, w_out):
    """Block-diagonal stationary operands (lhsT = W.T blocks) for 8 groups."""
    w0 = np.zeros((16, 128), dtype=np.float32)
    for g in range(G):
        w0[2 * g : 2 * g + 2, 16 * g : 16 * g + 16] = w_in.T  # [2,16]
    wm = np.zeros((N_INT, 128, 128), dtype=np.float32)
    for l in range(N_INT):
        for g in range(G):
            wm[l, 16 * g : 16 * g + 16, 16 * g : 16 * g + 16] = w_mid[l].T
    w6 = np.zeros((PACK, 128, 128), dtype=np.float32)
    for j in range(PACK):
        for g in range(G):
            w6[j, 16 * g : 16 * g + 16, 32 * j + 2 * g : 32 * j + 2 * g + 2] = (
                w_out.T
            )  # [16,2]
    return w0, wm, w6


def _shard_x(shard):
    """[N_SHARD, 2] -> [16, S*C] with row 2g+f, col s*C+c."""
    v = shard.reshape(S, G, C, 2)           # [s, g, c, f]
    v = v.transpose(1, 3, 0, 2)             # [g, f, s, c]
    return np.ascontiguousarray(v.reshape(16, S * C))


def _unshard_y(yd):
    """[128, (S//PACK)*C] -> [N_SHARD, 2]."""
    # row q = 32*j + 2*g + f with g in [0,8); rows q%32 >= 16 are unused.
    v = yd.reshape(PACK, 32, S // PACK, C)[:, :16]    # [j, 2g+f, sp, c]
    v = v.reshape(PACK, 8, 2, S // PACK, C)           # [j, g, f, sp, c]
    v = v.transpose(3, 0, 1, 4, 2)                    # [sp, j, g, c, f]
    return v.reshape(N_SHARD, 2)


def kernel(x, w_in, w_mid, w_out):
    from concourse.bass_utils import run_bass_kernel_spmd

    x = np.ascontiguousarray(x, dtype=np.float32)
    w0, wm, w6 = _prep_weights(
        np.asarray(w_in, dtype=np.float32),
        np.asarray(w_mid, dtype=np.float32),
        np.asarray(w_out, dtype=np.float32),
    )

    if "nc" not in _CACHE:
        _CACHE["nc"] = _build_bass()
    nc = _CACHE["nc"]

    in_maps = []
    for c in range(N_CORES):
        shard = x[c * N_SHARD : (c + 1) * N_SHARD]
        in_maps.append({"xd": _shard_x(shard), "w0": w0, "wm": wm, "w6": w6})

    res = run_bass_kernel_spmd(
        nc, in_maps, core_ids=list(range(N_CORES)), trace=TRACE
    )
    global LAST_RESULT
    LAST_RESULT = res
    y = np.empty((N, 2), dtype=np.float32)
    for c in range(N_CORES):
        y[c * N_SHARD : (c + 1) * N_SHARD] = _unshard_y(res.results[c]["yd"])
    return y


# revision 11
# speedup vs baseline: 1.3473x; 1.3473x over previous
"""Trainium2 Bass kernel for nn_Denoiser (dense MLP 2->16->16x5->2, N=4194304).

Strategy (pure data parallel over 8 NeuronCores):
  - Shard the batch over 8 cores (524288 points each); weights replicated.
  - On each core, stack 8 batch groups along SBUF partitions and use
    block-diagonal weights so each fp32 matmul column carries 8 points
    (the 128x128 PE array runs 8 independent 16-wide MLPs at once).
  - Activations are [128, 1024] fp32 tiles (features x batch-columns);
    each layer is 2 matmuls (PSUM bank = 512 fp32 cols); the 6 inner
    ReLUs are fused into the PSUM->SBUF evacuation, split between
    VectorE (tensor_scalar_max) and ScalarE (activation Relu).
  - The final 16->2 layer accumulates 4 super-chunks into one packed
    [128, 1024] PSUM tile (weights block-shifted by 32 partitions per
    super-chunk) so a single PSUM->SBUF copy serves 4 super-chunks.
  - The host pre-permutes x into the exact per-partition layout
    ([16, S*C], partition = 2*group+feature) so every device DMA is a
    contiguous 2D slice; the packed output [128, (S/4)*C] is decoded on
    the host the same way.
"""

import numpy as np

N = 4194304
N_CORES = 8
N_SHARD = N // N_CORES  # 524288
G = 8          # batch groups stacked along partitions
C = 1024       # batch columns per super-chunk tile (2 PSUM banks)
S = N_SHARD // (G * C)  # 64 super-chunks per core
PACK = 4       # super-chunks packed per output evacuation
N_NODE = 16
N_INT = 5

_CACHE = {}

# Set by test harnesses: TRACE=True captures an NTFF profile; the
# BassKernelResults of the last run lands in LAST_RESULT.
TRACE = False
LAST_RESULT = None


def _build_bass():
    from contextlib import ExitStack

    import concourse.mybir as mybir
    import concourse.tile as tile
    from concourse import bacc

    f32 = mybir.dt.float32
    f32r = mybir.dt.float32r
    nc = bacc.Bacc("TRN2", target_bir_lowering=False, num_devices=N_CORES)

    # xd[2g+f, s*C + c] = x[s*G*C + g*C + c, f]   (host pre-permuted)
    xd = nc.dram_tensor("xd", [16, S * C], f32r, kind="ExternalInput")
    w0 = nc.dram_tensor("w0", [16, 128], f32r, kind="ExternalInput")
    wm = nc.dram_tensor("wm", [N_INT, 128, 128], f32r, kind="ExternalInput")
    w6 = nc.dram_tensor("w6", [PACK, 128, 128], f32r, kind="ExternalInput")
    # yd[32*(s%4)+2g+f, (s//4)*C + c] = y[s*G*C + g*C + c, f]
    yd = nc.dram_tensor("yd", [128, (S // PACK) * C], f32, kind="ExternalOutput")

    with tile.TileContext(nc) as tc, ExitStack() as ctx:
        wpool = ctx.enter_context(tc.tile_pool(name="weights", bufs=1))
        xpool = ctx.enter_context(tc.tile_pool(name="x", bufs=3))
        hpool = ctx.enter_context(tc.tile_pool(name="h", bufs=3))
        opool = ctx.enter_context(tc.tile_pool(name="o", bufs=2))
        pspool = ctx.enter_context(tc.tile_pool(name="ps", bufs=3, space="PSUM"))
        pkpool = ctx.enter_context(tc.tile_pool(name="pk", bufs=1, space="PSUM"))

        w0_t = wpool.tile([16, 128], f32r, tag="w0")
        nc.sync.dma_start(out=w0_t, in_=w0[:, :])
        wm_t = []
        for l in range(N_INT):
            t = wpool.tile([128, 128], f32r, tag=f"wm{l}")
            nc.sync.dma_start(out=t, in_=wm[l, :, :])
            wm_t.append(t)
        w6_t = []
        for j in range(PACK):
            t = wpool.tile([128, 128], f32r, tag=f"w6{j}")
            nc.sync.dma_start(out=t, in_=w6[j, :, :])
            w6_t.append(t)

        pk_t = None
        for s in range(S):
            j = s % PACK
            x_t = xpool.tile([16, C], f32r, tag="x")
            nc.sync.dma_start(out=x_t, in_=xd[:, s * C : (s + 1) * C])
            h = x_t
            for l in range(6):
                lhsT = w0_t if l == 0 else wm_t[l - 1]
                ps_t = pspool.tile([128, C], f32, tag="ps")
                for q in range(C // 512):
                    nc.tensor.matmul(
                        ps_t[:, 512 * q : 512 * (q + 1)],
                        lhsT,
                        h[:, 512 * q : 512 * (q + 1)],
                        start=True,
                        stop=True,
                    )
                h_new = hpool.tile([128, C], f32r, tag="h")
                # ~44/56 DVE/ACT split balances the two evacuation engines.
                if (s + l) % 9 < 4:
                    nc.vector.tensor_scalar_max(h_new, ps_t, 0.0)
                else:
                    nc.scalar.activation(
                        h_new, ps_t, mybir.ActivationFunctionType.Relu
                    )
                h = h_new
            if j == 0:
                pk_t = pkpool.tile([128, C], f32, tag="pk")
            for q in range(C // 512):
                nc.tensor.matmul(
                    pk_t[:, 512 * q : 512 * (q + 1)],
                    w6_t[j],
                    h[:, 512 * q : 512 * (q + 1)],
                    start=(j == 0),
                    stop=(j == PACK - 1),
                    skip_group_check=True,
                )
            if j == PACK - 1:
                sp = s // PACK
                o_t = opool.tile([128, C], f32, tag="o")
                nc.any.tensor_copy(o_t, pk_t)
                nc.sync.dma_start(out=yd[:, sp * C : (sp + 1) * C], in_=o_t)
    nc.compile()
    return nc


def _prep_weights(w_in, w_mid, w_out):
    """Block-diagonal stationary operands (lhsT = W.T blocks) for 8 groups."""
    w0 = np.zeros((16, 128), dtype=np.float32)
    for g in range(G):
        w0[2 * g : 2 * g + 2, 16 * g : 16 * g + 16] = w_in.T  # [2,16]
    wm = np.zeros((N_INT, 128, 128), dtype=np.float32)
    for l in range(N_INT):
        for g in range(G):
            wm[l, 16 * g : 16 * g + 16, 16 * g : 16 * g + 16] = w_mid[l].T
    w6 = np.zeros((PACK, 128, 128), dtype=np.float32)
    for j in range(PACK):
        for g in range(G):
            w6[j, 16 * g : 16 * g + 16, 32 * j + 2 * g : 32 * j + 2 * g + 2] = (
                w_out.T
            )  # [16,2]
    return w0, wm, w6


def _shard_x(shard):
    """[N_SHARD, 2] -> [16, S*C] with row 2g+f, col s*C+c."""
    v = shard.reshape(S, G, C, 2)           # [s, g, c, f]
    v = v.transpose(1, 3, 0, 2)             # [g, f, s, c]
    return np.ascontiguousarray(v.reshape(16, S * C))


def _unshard_y(yd):
    """[128, (S//PACK)*C] -> [N_SHARD, 2]."""
    # row q = 32*j + 2*g + f with g in [0,8); rows q%32 >= 16 are unused.
    v = yd.reshape(PACK, 32, S // PACK, C)[:, :16]    # [j, 2g+f, sp, c]
    v = v.reshape(PACK, 8, 2, S // PACK, C)           # [j, g, f, sp, c]
    v = v.transpose(3, 0, 1, 4, 2)                    # [sp, j, g, c, f]
    return v.reshape(N_SHARD, 2)


def kernel(x, w_in, w_mid, w_out):
    from concourse.bass_utils import run_bass_kernel_spmd

    x = np.ascontiguousarray(x, dtype=np.float32)
    w0, wm, w6 = _prep_weights(
        np.asarray(w_in, dtype=np.float32),
        np.asarray(w_mid, dtype=np.float32),
        np.asarray(w_out, dtype=np.float32),
    )

    if "nc" not in _CACHE:
        _CACHE["nc"] = _build_bass()
    nc = _CACHE["nc"]

    in_maps = []
    for c in range(N_CORES):
        shard = x[c * N_SHARD : (c + 1) * N_SHARD]
        in_maps.append({"xd": _shard_x(shard), "w0": w0, "wm": wm, "w6": w6})

    res = run_bass_kernel_spmd(
        nc, in_maps, core_ids=list(range(N_CORES)), trace=TRACE
    )
    global LAST_RESULT
    LAST_RESULT = res
    y = np.empty((N, 2), dtype=np.float32)
    for c in range(N_CORES):
        y[c * N_SHARD : (c + 1) * N_SHARD] = _unshard_y(res.results[c]["yd"])
    return y
